# revision 25
# baseline (speedup 1.0000x reference)
"""Trainium2 Bass kernel for nn_MixtureOfHMM.

Math: the per-step emission logprob e_t[b] = emit[b, x[b,t]] is identical
across all (mixture, state) pairs, so the HMM recurrence
    z_t = LSE_prev(logT + z_{t-1}) + e_t
splits into z_t = w_t + sum_{t'<=t} e_{t'} with a data-independent carry
    w_t = LSE_prev(logT + w_{t-1}),  w_0 = log_softmax(init/2).
Hence
    out[b] = K + S1[b]/T - L[b]
      K    = LSE_{m,s}(w_T[m,s] / T)                  (from init/transition only)
      S1[b]= sum_g counts[b,g] * logits[b,g]
      L[b] = LSE_g logits[b,g]
      logits = (counts @ embed_table)/T @ vocab_w.T + vocab_b
K is computed on host (4 MFLOP, log-semiring matrix squaring).

Device work is split into two collective-free SPMD launches on 8 cores
(the on-device AllReduce path costs ~40us of barrier+mesh overhead in
this environment, far more than a second launch):
  A: per-core partial mean over its vocab shard (compact: only embed
     rows actually referenced by x are shipped), host sums 8x[16,512].
  B: logits = mean @ vocab_wT + vb over the core's vocab shard, then
     sum exp / sum counts*logits partials per (quadrant, batch).
Host combines all per-core partials exactly (log-sum-exp merge).

Perf notes (vs the 50us baseline):
  - All launch inputs are packed on host into partition-contiguous DRAM
    tensors so every DMA descriptor is one large contiguous read per
    partition (the baseline's strided layouts produced 32B-1KB packets
    and ~2x DMA stalls).
  - Launch B accumulates all logits into a single [128, 1000] PSUM tile
    (matmul tile_position quadrant bases 0/32/64/96); the log-softmax
    reductions read PSUM directly, removing 8 serial PSUM->SBUF copies.
  - The vocab bias is injected with 4 tiny ones x vb matmuls that run in
    the DMA shadow and double as PE clock warmup; 1/32 fp8 scaling is
    folded into the exp() scale and the host-side counts tensor.
  - A dummy exp() early in launch B pulls the ACT_TABLE_LOAD off the
    critical path.
"""

import os
import sys

import numpy as np

for _p in ("/opt/trn_rl_repo", "/root/.axon_site/_ro/trn_rl_repo"):
    if os.path.isdir(_p) and _p not in sys.path:
        sys.path.insert(0, _p)

import concourse.bacc as bacc
import concourse.mybir as mybir
import concourse.tile as tile
from concourse import bass_utils

B, T = 16, 1024
G, E = 32000, 512
NC = 8
GS = G // NC            # 4000 vocab rows per core
GSUB = 8                # vocab sub-blocks (quadrant q = gs//2, half h = gs%2)
GBLK = GS // GSUB       # 500
CHB = 2 * E + 2 * B     # 1056 bytes per DoubleRow chunk per partition (A)
DEF_GSP = 2048
NJ_A = 6                # junk warmup matmuls in launch A
NJ_B = 2                # junk warmup matmuls in launch B

_prog_cache = {}


def _new_bass():
    return bacc.Bacc(
        "TRN2",
        target_bir_lowering=False,
        debug=False,
        enable_asserts=True,
        num_devices=NC,
    )


def _build_program_a(gsp=DEF_GSP):
    """Partial mean (x T): pmean[b,e] = sum_g counts[b,g] * embed[g,e].

    Input xa is host-packed [128, kch*1056] fp8: partition p, chunk k
    carries the embed-row pair (256k+2p, 256k+2p+1) as 2x512B followed by
    the matching raw-count pair as 2x16B, so each partition's DMA is one
    contiguous descriptor and the DoubleRow matmul reads both operands
    from the same tile.  Raw counts are exact in fp8; host divides by T.
    """
    kch = gsp // 256
    # 2 DMA groups (larger descriptors stream faster; a finer split was
    # measured slower end-to-end)
    g1 = (kch + 1) // 2
    sizes = [s for s in (g1, kch - g1) if s > 0]
    starts = [sum(sizes[:i]) for i in range(len(sizes))]
    f32 = mybir.dt.float32
    bf16 = mybir.dt.bfloat16
    f8 = mybir.dt.float8e4
    nc = _new_bass()
    xa = nc.dram_tensor("xa", [128, kch * CHB], f8, kind="ExternalInput")
    outm = nc.dram_tensor("outm", [B, E], bf16, kind="ExternalOutput")

    with tile.TileContext(nc) as tc:
        with (
            tc.tile_pool(name="sb", bufs=1) as sb,
            tc.tile_pool(name="ps", bufs=1, space="PSUM") as ps,
        ):
            xts = []
            for i, (s0, sz) in enumerate(zip(starts, sizes)):
                xt = sb.tile([128, sz * CHB], f8, tag=f"xt{i}")
                nc.sync.dma_start(
                    out=xt[:], in_=xa.ap()[:, s0 * CHB : (s0 + sz) * CHB]
                )
                xts.append(xt)
            # PE warmup while the DMA streams: HAM un-throttles only after
            # sustained PE activity, so burn the wait on junk matmuls.
            wj = sb.tile([128, E], f8, tag="wj")
            nc.vector.memset(wj[:], 0.0)
            wp = ps.tile([128, E], f32, tag="wp")
            for _ in range(NJ_A):
                nc.tensor.matmul(
                    wp[:], wj[:, 0:128], wj[:],
                    start=True, stop=False, skip_group_check=True,
                )
            pm = ps.tile([B, E], f32, tag="pmean")
            for k in range(kch):
                gi = next(i for i in range(len(sizes))
                          if starts[i] <= k < starts[i] + sizes[i])
                t, off = xts[gi], (k - starts[gi]) * CHB
                nc.tensor.matmul(
                    pm[:],
                    t[:, off + 2 * E : off + CHB].rearrange(
                        "p (r m) -> p r m", r=2
                    ),
                    t[:, off : off + 2 * E].rearrange("p (r e) -> p r e", r=2),
                    start=(k == 0),
                    stop=(k == kch - 1),
                    perf_mode=mybir.MatmulPerfMode.DoubleRow,
                )
            pmean_sb = sb.tile([B, E], bf16, tag="pmean_sb")
            nc.vector.tensor_copy(pmean_sb[:], pm[:])
            nc.sync.dma_start(out=outm.ap(), in_=pmean_sb[:])

    nc.compile()
    return nc


def _build_program_b():
    """logits over the core's vocab shard + log-softmax partials.

    ISA limits force one [B, 500] PSUM accumulator per vocab block gs
    (DoubleRow matmul dst must start at partition 0 and a matmul dst is
    capped at 512 free elements).  PSUM holds 32x(logits) (membT
    pre-scaled for fp8 range); the assembly copies into the [128, 1000]
    SBUF layout (quadrant q = gs//2 at partition base q*32, half h =
    gs%2 at free offset h*500) fold the 1/32 back out.  Copies are
    interleaved with the second k-chunk matmuls (per-block PSUM tiles,
    alternating scalar/vector) so only the last one trails the PE.
    The vocab bias is a single vector tensor_add of a host-replicated
    [128, 1000] vb tile (no PE matmuls -- the throttled PE is the
    bottleneck).  vw arrives as 4 DMA groups so the DR matmuls start as
    soon as the first quarter lands and drain during the stream.
    Rows +16..31 of each quadrant are never written (garbage) -- the
    host combine ignores them and cre is zero there.

    Outputs out[:, 0] = sum_g exp(logits), out[:, 1] = sum_g c*logits
    per (quadrant, batch) partition row.
    """
    f32 = mybir.dt.float32
    bf16 = mybir.dt.bfloat16
    f8 = mybir.dt.float8e4
    nc = _new_bass()
    # vwa: membT (64B) + vw k0 gs0-3; vwb: k0 gs4-7; vwc: k1 gs0-3;
    # vwd: k1 gs4-7 (last).  cv (vb replicated per quadrant row +
    # counts) lands 4th so the vector copies have the bias in time.
    # All descriptors are >= 4000B/partition: 2000B descriptors were
    # measured to stream at only ~190 GB/s vs ~330 for 4000B.
    vwa = nc.dram_tensor("vwa", [128, 64 + 4000], f8, kind="ExternalInput")
    vwb = nc.dram_tensor("vwb", [128, 4000], f8, kind="ExternalInput")
    vwc = nc.dram_tensor("vwc", [128, 4000], f8, kind="ExternalInput")
    vwd = nc.dram_tensor("vwd", [128, 4000], f8, kind="ExternalInput")
    cv = nc.dram_tensor("cv", [128, 4 * GBLK], bf16, kind="ExternalInput")
    out = nc.dram_tensor("out", [128, 2], f32, kind="ExternalOutput")

    with tile.TileContext(nc) as tc:
        with (
            tc.tile_pool(name="sb", bufs=1) as sb,
            tc.tile_pool(name="ps", bufs=1, space="PSUM") as ps,
        ):
            vwa_sb = sb.tile([128, 64 + 4000], f8, tag="vwa")
            nc.sync.dma_start(out=vwa_sb[:], in_=vwa.ap())
            vwb_sb = sb.tile([128, 4000], f8, tag="vwb")
            nc.sync.dma_start(out=vwb_sb[:], in_=vwb.ap())
            vwc_sb = sb.tile([128, 4000], f8, tag="vwc")
            nc.sync.dma_start(out=vwc_sb[:], in_=vwc.ap())
            cv_sb = sb.tile([128, 4 * GBLK], bf16, tag="cv")
            nc.sync.dma_start(out=cv_sb[:], in_=cv.ap())
            vwd_sb = sb.tile([128, 4000], f8, tag="vwd")
            nc.sync.dma_start(out=vwd_sb[:], in_=vwd.ap())

            # preload the exp() activation table off the critical path
            dmy = sb.tile([1, 1], f32, tag="dmy")
            nc.vector.memset(dmy[:], 0.0)
            dmy2 = sb.tile([1, 1], f32, tag="dmy2")
            nc.scalar.activation(
                dmy2[:], dmy[:], mybir.ActivationFunctionType.Exp,
                bias=0.0, scale=1.0,
            )

            wj = sb.tile([128, 512], f8, tag="wj")
            nc.vector.memset(wj[:], 0.0)
            plgs = [
                ps.tile([B, GBLK], f32, tag=f"plg{gs}", name=f"plg{gs}")
                for gs in range(GSUB)
            ]
            # PE warmup into plg0's bank (its start=True k0 matmul below
            # overwrites the junk).
            for _ in range(NJ_B):
                nc.tensor.matmul(
                    plgs[0][:], wj[:, 0:B], wj[:, 0:GBLK],
                    start=True, stop=False, skip_group_check=True,
                )
            # DoubleRow fp8: partition p carries rows e = k*256 + 2p + r.
            membT_v = vwa_sb[:, 0:64].rearrange("p (k r m) -> p k r m", k=2, r=2)
            srcs = {(0, 0): (vwa_sb, 64), (0, 1): (vwa_sb, 64),
                    (0, 2): (vwb_sb, -4000), (0, 3): (vwb_sb, -4000),
                    (1, 0): (vwc_sb, 0), (1, 1): (vwc_sb, 0),
                    (1, 2): (vwd_sb, -4000), (1, 3): (vwd_sb, -4000)}
            for k in range(2):
                for gs in range(GSUB):
                    src, base = srcs[(k, gs // 2)]
                    off = base + gs * 1000
                    nc.tensor.matmul(
                        plgs[gs][:],
                        membT_v[:, k],
                        src[:, off : off + 1000].rearrange("p (r g) -> p r g", r=2),
                        start=(k == 0),
                        stop=(k == 1),
                        perf_mode=mybir.MatmulPerfMode.DoubleRow,
                        skip_group_check=(gs == 0 and k == 0),
                    )
            # assemble true logits into [128, 1000] bf16; per-block PSUM
            # tiles let each copy chase its own stop matmul.  Vector-side
            # copies (h=1 cols) fold in 1/32 AND the vocab bias; scalar-side
            # copies (h=0) fold 1/32 only and one half-width vector add
            # applies their bias afterwards.
            lgb_sb = sb.tile([128, 2 * GBLK], bf16, tag="lgb_sb")
            lgs_sb = sb.tile([128, GBLK], bf16, tag="lgs_sb")
            for gs in range(GSUB):
                q, h = gs // 2, gs % 2
                if h == 0:
                    nc.scalar.mul(
                        lgs_sb[:][q * 32 : q * 32 + B, :], plgs[gs][:], 1.0 / 32.0
                    )
                else:
                    nc.vector.scalar_tensor_tensor(
                        lgb_sb[:][q * 32 : q * 32 + B, GBLK : 2 * GBLK],
                        plgs[gs][:],
                        1.0 / 32.0,
                        cv_sb[:][q * 32 : q * 32 + B, GBLK : 2 * GBLK],
                        op0=mybir.AluOpType.mult,
                        op1=mybir.AluOpType.add,
                    )
            nc.vector.tensor_add(
                lgb_sb[:, 0:GBLK], lgs_sb[:], cv_sb[:, 0:GBLK]
            )
            # reductions: col0 = sum exp(logits), col1 = sum counts*logits
            out_sb = sb.tile([128, 2], f32, tag="out_sb")
            scr_e = sb.tile([128, 2 * GBLK], bf16, tag="scr_e")
            scr_m = sb.tile([128, 2 * GBLK], bf16, tag="scr_m")
            nc.scalar.activation(
                scr_e[:],
                lgb_sb[:],
                mybir.ActivationFunctionType.Exp,
                bias=0.0,
                scale=1.0,
                accum_out=out_sb[:, 0:1],
            )
            nc.vector.scalar_tensor_tensor(
                scr_m[:],
                lgb_sb[:],
                1.0,
                cv_sb[:, 2 * GBLK : 4 * GBLK],
                op0=mybir.AluOpType.mult,
                op1=mybir.AluOpType.mult,
                accum_out=out_sb[:, 1:2],
            )
            nc.sync.dma_start(out=out.ap(), in_=out_sb[:])

    nc.compile()
    return nc


def _get_program_a(gsp=DEF_GSP):
    key = ("a", gsp)
    if key not in _prog_cache:
        _prog_cache[key] = _build_program_a(gsp)
    return _prog_cache[key]


def _get_program_b():
    if "b" not in _prog_cache:
        _prog_cache["b"] = _build_program_b()
    return _prog_cache["b"]


def _hmm_const(init_dist, transition):
    """K = LSE_{m,s}(w_T/T) via log-semiring matrix powering (float64)."""
    init = np.asarray(init_dist, np.float64)[0]      # [M,S]
    tr = np.asarray(transition, np.float64)[0]       # [M,S,S]
    a = init / 2.0
    m_ = a.max(axis=1, keepdims=True)
    z0 = a - (m_ + np.log(np.exp(a - m_).sum(axis=1, keepdims=True)))
    a = tr / 2.0
    m_ = a.max(axis=1, keepdims=True)
    logT = a - (m_ + np.log(np.exp(a - m_).sum(axis=1, keepdims=True)))

    mix = z0.shape[0]
    v = np.exp(z0)                                   # [M,S]
    vlog = np.zeros(mix)
    P = np.exp(logT)                                 # [M,S,S]
    plog = np.zeros(mix)
    n = T
    while n:
        if n & 1:
            v = np.einsum("ms,mst->mt", v, P)
            vlog += plog
            s = v.max(axis=1)
            v /= s[:, None]
            vlog += np.log(s)
        n >>= 1
        if n:
            P = np.einsum("mst,mtu->msu", P, P)
            plog *= 2
            s = P.max(axis=(1, 2))
            P /= s[:, None, None]
            plog += np.log(s)
    w = (np.log(v) + vlog[:, None]) / T              # [M,S]
    mx = w.max()
    return mx + np.log(np.exp(w - mx).sum())


def _counts_from_x(x):
    counts = np.zeros((B, G), np.float32)
    for b in range(B):
        counts[b] = np.bincount(np.asarray(x[b], np.int64), minlength=G)
    return counts


def _prep_in_maps_a(counts, embed_table_f8):
    """Compact phase-1 inputs: only referenced embed rows matter for the
    counts contraction; gathering them on host (pure index marshalling)
    lets the device read ~40% of the shard.  Rows are packed with their
    counts into one partition-contiguous tensor per core."""
    shard_cols = []
    nu_max = 0
    for c in range(NC):
        cols = np.nonzero(counts[:, c * GS : (c + 1) * GS].sum(axis=0))[0]
        shard_cols.append(cols)
        nu_max = max(nu_max, len(cols))
    gsp = max(512, -(-nu_max // 256) * 256)
    kch = gsp // 256

    import ml_dtypes

    f8 = ml_dtypes.float8_e4m3fn
    in_maps = []
    for c in range(NC):
        g0 = c * GS
        cols = shard_cols[c]
        emb_pad = np.zeros((gsp, E), f8)
        emb_pad[: len(cols)] = embed_table_f8[g0 + cols]
        ctT = np.zeros((gsp, B), f8)
        # raw counts are small ints, exact in fp8
        ctT[: len(cols)] = counts[:, g0 : g0 + GS][:, cols].T.astype(f8)
        # [kch, 128, 2*E] and [kch, 128, 2*B]: row index = k*256 + 2p + r
        emb_r = emb_pad.reshape(kch, 128, 2 * E)
        ct_r = ctT.reshape(kch, 128, 2 * B)
        xa = np.concatenate([emb_r, ct_r], axis=2)   # [kch, 128, CHB]
        xa = np.ascontiguousarray(xa.transpose(1, 0, 2)).reshape(128, kch * CHB)
        in_maps.append({"xa": xa})
    return in_maps, gsp


def _prep_in_maps_b(counts, mean_emb, vocab_w_f8, vocab_b_f32):
    import ml_dtypes

    f8 = ml_dtypes.float8_e4m3fn
    bf16 = ml_dtypes.bfloat16
    # membT[p, k*32 + r*16 + m] = 32*mean_emb[m, k*256 + 2p + r]
    met = (mean_emb * 32.0).T.reshape(2, 128, 2, B)      # [k, p, r, m]
    membT = np.ascontiguousarray(met.transpose(1, 0, 2, 3).reshape(128, 4 * B)).astype(f8)
    in_maps = []
    for c in range(NC):
        g0, g1 = c * GS, (c + 1) * GS
        # vw_dr[p, gs*1000 + r*500 + j] = vocab_w[g0 + gs*500 + j, k*256 + 2p + r]
        v = vocab_w_f8[g0:g1].T.reshape(2, 128, 2, GSUB, GBLK)   # [k, p, r, gs, j]
        vk = np.ascontiguousarray(v.transpose(1, 0, 3, 2, 4)).reshape(128, 2, 8000)
        vwa = np.concatenate([membT, vk[:, 0, :4000]], axis=1)   # [128, 4064]
        vwb = np.ascontiguousarray(vk[:, 0, 4000:])              # [128, 4000]
        vwc = np.ascontiguousarray(vk[:, 1, :4000])
        vwd = np.ascontiguousarray(vk[:, 1, 4000:])
        # cv[:, :1000]: vb replicated per quadrant row; cv[:, 1000:]: counts
        # (both in the [q*32+b, h*500+j] <-> g0 + (2q+h)*500 + j layout)
        vbq = vocab_b_f32[g0:g1].reshape(4, 2 * GBLK)
        cq = counts[:, g0:g1].reshape(B, 4, 2 * GBLK).transpose(1, 0, 2)
        cv = np.zeros((128, 4 * GBLK), bf16)
        for q in range(4):
            cv[q * 32 : (q + 1) * 32, : 2 * GBLK] = vbq[q].astype(bf16)
            cv[q * 32 : q * 32 + B, 2 * GBLK :] = cq[q].astype(bf16)
        in_maps.append(
            {"vwa": vwa, "vwb": vwb, "vwc": vwc, "vwd": vwd, "cv": cv}
        )
    return in_maps


def _combine(core_outs, K):
    """Exact host-side combine of the per-(core, quadrant, b) partials."""
    sumexp = np.empty((NC, 4, B), np.float64)
    s1 = np.empty((NC, 4, B), np.float64)
    for c in range(NC):
        o = np.asarray(core_outs[c], np.float64).reshape(4, 32, 2)[:, :B]
        sumexp[c] = o[:, :, 0]
        s1[c] = o[:, :, 1]
    L = np.log(sumexp.sum(axis=(0, 1)))              # [B] (logits ~ +-0.2, safe)
    S1 = s1.sum(axis=(0, 1))
    out = K + S1 / T - L
    return out.astype(np.float32).reshape(B, 1)


def kernel(**inputs):
    import ml_dtypes

    f8 = ml_dtypes.float8_e4m3fn
    K = _hmm_const(inputs["init_dist"], inputs["transition"])
    counts = _counts_from_x(np.asarray(inputs["x"]))
    embed_table = np.asarray(inputs["embed_table"], np.float32).astype(f8)
    vocab_w = np.asarray(inputs["vocab_w"], np.float32).astype(f8)
    vocab_b = np.asarray(inputs["vocab_b"], np.float32)

    in_maps_a, gsp = _prep_in_maps_a(counts, embed_table)
    res_a = bass_utils.run_bass_kernel_spmd(
        _get_program_a(gsp), in_maps_a, core_ids=list(range(NC))
    )
    mean_emb = np.zeros((B, E), np.float64)
    for r in res_a.results:
        mean_emb += np.asarray(r["outm"], np.float64)
    mean_emb = (mean_emb / T).astype(np.float32)

    in_maps_b = _prep_in_maps_b(counts, mean_emb, vocab_w, vocab_b)
    res_b = bass_utils.run_bass_kernel_spmd(
        _get_program_b(), in_maps_b, core_ids=list(range(NC))
    )
    return _combine([r["out"] for r in res_b.results], K)


# revision 27
# speedup vs baseline: 1.0241x; 1.0241x over previous
"""Trainium2 Bass kernel for nn_MixtureOfHMM.

Math: the per-step emission logprob e_t[b] = emit[b, x[b,t]] is identical
across all (mixture, state) pairs, so the HMM recurrence
    z_t = LSE_prev(logT + z_{t-1}) + e_t
splits into z_t = w_t + sum_{t'<=t} e_{t'} with a data-independent carry
    w_t = LSE_prev(logT + w_{t-1}),  w_0 = log_softmax(init/2).
Hence
    out[b] = K + S1[b]/T - L[b]
      K    = LSE_{m,s}(w_T[m,s] / T)                  (from init/transition only)
      S1[b]= sum_g counts[b,g] * logits[b,g]
      L[b] = LSE_g logits[b,g]
      logits = (counts @ embed_table)/T @ vocab_w.T + vocab_b
K is computed on host (4 MFLOP, log-semiring matrix squaring).

Device work is split into two collective-free SPMD launches on 8 cores
(the on-device AllReduce path costs ~40us of barrier+mesh overhead in
this environment, far more than a second launch):
  A: per-core partial mean over its vocab shard (compact: only embed
     rows actually referenced by x are shipped), host sums 8x[16,512].
  B: logits = mean @ vocab_wT + vb over the core's vocab shard, then
     sum exp / sum counts*logits partials per (quadrant, batch).
Host combines all per-core partials exactly (log-sum-exp merge).

Perf notes (vs the 50us baseline):
  - All launch inputs are packed on host into partition-contiguous DRAM
    tensors so every DMA descriptor is one large contiguous read per
    partition (the baseline's strided layouts produced 32B-1KB packets
    and ~2x DMA stalls).
  - Launch B accumulates all logits into a single [128, 1000] PSUM tile
    (matmul tile_position quadrant bases 0/32/64/96); the log-softmax
    reductions read PSUM directly, removing 8 serial PSUM->SBUF copies.
  - The vocab bias is injected with 4 tiny ones x vb matmuls that run in
    the DMA shadow and double as PE clock warmup; 1/32 fp8 scaling is
    folded into the exp() scale and the host-side counts tensor.
  - A dummy exp() early in launch B pulls the ACT_TABLE_LOAD off the
    critical path.
"""

import os
import sys

import numpy as np

for _p in ("/opt/trn_rl_repo", "/root/.axon_site/_ro/trn_rl_repo"):
    if os.path.isdir(_p) and _p not in sys.path:
        sys.path.insert(0, _p)

import concourse.bacc as bacc
import concourse.mybir as mybir
import concourse.tile as tile
from concourse import bass_utils

B, T = 16, 1024
G, E = 32000, 512
NC = 8
GS = G // NC            # 4000 vocab rows per core
GSUB = 8                # vocab sub-blocks (quadrant q = gs//2, half h = gs%2)
GBLK = GS // GSUB       # 500
CHB = 2 * E + 2 * B     # 1056 bytes per DoubleRow chunk per partition (A)
DEF_GSP = 2048
NJ_A = 6                # junk warmup matmuls in launch A
NJ_B = 2                # junk warmup matmuls in launch B

_prog_cache = {}


def _new_bass():
    return bacc.Bacc(
        "TRN2",
        target_bir_lowering=False,
        debug=False,
        enable_asserts=True,
        num_devices=NC,
    )


def _build_program_a(gsp=DEF_GSP):
    """Partial mean (x T): pmean[b,e] = sum_g counts[b,g] * embed[g,e].

    Input xa is host-packed [128, kch*1056] fp8: partition p, chunk k
    carries the embed-row pair (256k+2p, 256k+2p+1) as 2x512B followed by
    the matching raw-count pair as 2x16B, so each partition's DMA is one
    contiguous descriptor and the DoubleRow matmul reads both operands
    from the same tile.  Raw counts are exact in fp8; host divides by T.
    """
    kch = gsp // 256
    # 2 DMA groups (larger descriptors stream faster; a finer split was
    # measured slower end-to-end)
    g1 = (kch + 1) // 2
    sizes = [s for s in (g1, kch - g1) if s > 0]
    starts = [sum(sizes[:i]) for i in range(len(sizes))]
    f32 = mybir.dt.float32
    bf16 = mybir.dt.bfloat16
    f8 = mybir.dt.float8e4
    nc = _new_bass()
    xa = nc.dram_tensor("xa", [128, kch * CHB], f8, kind="ExternalInput")
    outm = nc.dram_tensor("outm", [B, E], bf16, kind="ExternalOutput")

    with tile.TileContext(nc) as tc:
        with (
            tc.tile_pool(name="sb", bufs=1) as sb,
            tc.tile_pool(name="ps", bufs=1, space="PSUM") as ps,
        ):
            xts = []
            for i, (s0, sz) in enumerate(zip(starts, sizes)):
                xt = sb.tile([128, sz * CHB], f8, tag=f"xt{i}")
                nc.sync.dma_start(
                    out=xt[:], in_=xa.ap()[:, s0 * CHB : (s0 + sz) * CHB]
                )
                xts.append(xt)
            # PE warmup while the DMA streams: HAM un-throttles only after
            # sustained PE activity, so burn the wait on junk matmuls.
            wj = sb.tile([128, E], f8, tag="wj")
            nc.vector.memset(wj[:], 0.0)
            wp = ps.tile([128, E], f32, tag="wp")
            for _ in range(NJ_A):
                nc.tensor.matmul(
                    wp[:], wj[:, 0:128], wj[:],
                    start=True, stop=False, skip_group_check=True,
                )
            pm = ps.tile([B, E], f32, tag="pmean")
            for k in range(kch):
                gi = next(i for i in range(len(sizes))
                          if starts[i] <= k < starts[i] + sizes[i])
                t, off = xts[gi], (k - starts[gi]) * CHB
                nc.tensor.matmul(
                    pm[:],
                    t[:, off + 2 * E : off + CHB].rearrange(
                        "p (r m) -> p r m", r=2
                    ),
                    t[:, off : off + 2 * E].rearrange("p (r e) -> p r e", r=2),
                    start=(k == 0),
                    stop=(k == kch - 1),
                    perf_mode=mybir.MatmulPerfMode.DoubleRow,
                )
            pmean_sb = sb.tile([B, E], bf16, tag="pmean_sb")
            nc.vector.tensor_copy(pmean_sb[:], pm[:])
            nc.sync.dma_start(out=outm.ap(), in_=pmean_sb[:])

    nc.compile()
    return nc


def _build_program_b():
    """logits over the core's vocab shard + log-softmax partials.

    ISA limits force one [B, 500] PSUM accumulator per vocab block gs
    (DoubleRow matmul dst must start at partition 0 and a matmul dst is
    capped at 512 free elements).  PSUM holds 32x(logits) (membT
    pre-scaled for fp8 range); the assembly copies into the [128, 1000]
    SBUF layout (quadrant q = gs//2 at partition base q*32, half h =
    gs%2 at free offset h*500) fold the 1/32 back out.  Copies are
    interleaved with the second k-chunk matmuls (per-block PSUM tiles,
    alternating scalar/vector) so only the last one trails the PE.
    The vocab bias is a single vector tensor_add of a host-replicated
    [128, 1000] vb tile (no PE matmuls -- the throttled PE is the
    bottleneck).  vw arrives as 4 DMA groups so the DR matmuls start as
    soon as the first quarter lands and drain during the stream.
    Rows +16..31 of each quadrant are never written (garbage) -- the
    host combine ignores them and cre is zero there.

    Outputs out[:, 0] = sum_g exp(logits), out[:, 1] = sum_g c*logits
    per (quadrant, batch) partition row.
    """
    f32 = mybir.dt.float32
    bf16 = mybir.dt.bfloat16
    f8 = mybir.dt.float8e4
    nc = _new_bass()
    # vwa: membT (64B) + vw k0 gs0-3; vwb: k0 gs4-7; vwc: k1 gs0-3;
    # vwd: k1 gs4-7 (last).  cv (vb replicated per quadrant row +
    # counts) lands 4th so the vector copies have the bias in time.
    # All descriptors are >= 4000B/partition: 2000B descriptors were
    # measured to stream at only ~190 GB/s vs ~330 for 4000B.
    vwa = nc.dram_tensor("vwa", [128, 64 + 4000], f8, kind="ExternalInput")
    vwb = nc.dram_tensor("vwb", [128, 4000], f8, kind="ExternalInput")
    vwc = nc.dram_tensor("vwc", [128, 4000], f8, kind="ExternalInput")
    vwd = nc.dram_tensor("vwd", [128, 4000], f8, kind="ExternalInput")
    cv = nc.dram_tensor("cv", [128, 4 * GBLK], bf16, kind="ExternalInput")
    out = nc.dram_tensor("out", [128, 2], f32, kind="ExternalOutput")

    with tile.TileContext(nc) as tc:
        with (
            tc.tile_pool(name="sb", bufs=1) as sb,
            tc.tile_pool(name="ps", bufs=1, space="PSUM") as ps,
        ):
            vwa_sb = sb.tile([128, 64 + 4000], f8, tag="vwa")
            nc.sync.dma_start(out=vwa_sb[:], in_=vwa.ap())
            vwb_sb = sb.tile([128, 4000], f8, tag="vwb")
            nc.sync.dma_start(out=vwb_sb[:], in_=vwb.ap())
            vwc_sb = sb.tile([128, 4000], f8, tag="vwc")
            nc.sync.dma_start(out=vwc_sb[:], in_=vwc.ap())
            vwd_sb = sb.tile([128, 4000], f8, tag="vwd")
            nc.sync.dma_start(out=vwd_sb[:], in_=vwd.ap())
            # cv last: it is first needed by the bias add, well after the
            # final vw group's matmuls
            cv_sb = sb.tile([128, 4 * GBLK], bf16, tag="cv")
            nc.sync.dma_start(out=cv_sb[:], in_=cv.ap())

            # preload the exp() activation table off the critical path
            dmy = sb.tile([1, 1], f32, tag="dmy")
            nc.vector.memset(dmy[:], 0.0)
            dmy2 = sb.tile([1, 1], f32, tag="dmy2")
            nc.scalar.activation(
                dmy2[:], dmy[:], mybir.ActivationFunctionType.Exp,
                bias=0.0, scale=1.0,
            )

            wj = sb.tile([128, 512], f8, tag="wj")
            nc.vector.memset(wj[:], 0.0)
            plgs = [
                ps.tile([B, GBLK], f32, tag=f"plg{gs}", name=f"plg{gs}")
                for gs in range(GSUB)
            ]
            # PE warmup into plg0's bank (its start=True k0 matmul below
            # overwrites the junk).
            for _ in range(NJ_B):
                nc.tensor.matmul(
                    plgs[0][:], wj[:, 0:B], wj[:, 0:GBLK],
                    start=True, stop=False, skip_group_check=True,
                )
            # DoubleRow fp8: partition p carries rows e = k*256 + 2p + r.
            membT_v = vwa_sb[:, 0:64].rearrange("p (k r m) -> p k r m", k=2, r=2)
            srcs = {(0, 0): (vwa_sb, 64), (0, 1): (vwa_sb, 64),
                    (0, 2): (vwb_sb, -4000), (0, 3): (vwb_sb, -4000),
                    (1, 0): (vwc_sb, 0), (1, 1): (vwc_sb, 0),
                    (1, 2): (vwd_sb, -4000), (1, 3): (vwd_sb, -4000)}
            for k in range(2):
                for gs in range(GSUB):
                    src, base = srcs[(k, gs // 2)]
                    off = base + gs * 1000
                    nc.tensor.matmul(
                        plgs[gs][:],
                        membT_v[:, k],
                        src[:, off : off + 1000].rearrange("p (r g) -> p r g", r=2),
                        start=(k == 0),
                        stop=(k == 1),
                        perf_mode=mybir.MatmulPerfMode.DoubleRow,
                        skip_group_check=(gs == 0 and k == 0),
                    )
            # assemble true logits (x 1/32) into [128, 1000] bf16; per-block
            # PSUM tiles let each copy chase its own stop matmul.
            lgs_sb = sb.tile([128, 2 * GBLK], bf16, tag="lgs_sb")
            for gs in range(GSUB):
                q, h = gs // 2, gs % 2
                dst = lgs_sb[:][q * 32 : q * 32 + B, h * GBLK : (h + 1) * GBLK]
                if h == 0:
                    nc.scalar.mul(dst, plgs[gs][:], 1.0 / 32.0)
                else:
                    nc.vector.tensor_scalar_mul(dst, plgs[gs][:], 1.0 / 32.0)
            # one full-width add applies the vocab bias
            lgb_sb = sb.tile([128, 2 * GBLK], bf16, tag="lgb_sb")
            nc.vector.tensor_add(lgb_sb[:], lgs_sb[:], cv_sb[:, 0 : 2 * GBLK])
            # reductions: col0 = sum exp(logits), col1 = sum counts*logits
            out_sb = sb.tile([128, 2], f32, tag="out_sb")
            scr_e = sb.tile([128, 2 * GBLK], bf16, tag="scr_e")
            scr_m = sb.tile([128, 2 * GBLK], bf16, tag="scr_m")
            nc.scalar.activation(
                scr_e[:],
                lgb_sb[:],
                mybir.ActivationFunctionType.Exp,
                bias=0.0,
                scale=1.0,
                accum_out=out_sb[:, 0:1],
            )
            nc.vector.scalar_tensor_tensor(
                scr_m[:],
                lgb_sb[:],
                1.0,
                cv_sb[:, 2 * GBLK : 4 * GBLK],
                op0=mybir.AluOpType.mult,
                op1=mybir.AluOpType.mult,
                accum_out=out_sb[:, 1:2],
            )
            nc.sync.dma_start(out=out.ap(), in_=out_sb[:])

    nc.compile()
    return nc


def _get_program_a(gsp=DEF_GSP):
    key = ("a", gsp)
    if key not in _prog_cache:
        _prog_cache[key] = _build_program_a(gsp)
    return _prog_cache[key]


def _get_program_b():
    if "b" not in _prog_cache:
        _prog_cache["b"] = _build_program_b()
    return _prog_cache["b"]


def _hmm_const(init_dist, transition):
    """K = LSE_{m,s}(w_T/T) via log-semiring matrix powering (float64)."""
    init = np.asarray(init_dist, np.float64)[0]      # [M,S]
    tr = np.asarray(transition, np.float64)[0]       # [M,S,S]
    a = init / 2.0
    m_ = a.max(axis=1, keepdims=True)
    z0 = a - (m_ + np.log(np.exp(a - m_).sum(axis=1, keepdims=True)))
    a = tr / 2.0
    m_ = a.max(axis=1, keepdims=True)
    logT = a - (m_ + np.log(np.exp(a - m_).sum(axis=1, keepdims=True)))

    mix = z0.shape[0]
    v = np.exp(z0)                                   # [M,S]
    vlog = np.zeros(mix)
    P = np.exp(logT)                                 # [M,S,S]
    plog = np.zeros(mix)
    n = T
    while n:
        if n & 1:
            v = np.einsum("ms,mst->mt", v, P)
            vlog += plog
            s = v.max(axis=1)
            v /= s[:, None]
            vlog += np.log(s)
        n >>= 1
        if n:
            P = np.einsum("mst,mtu->msu", P, P)
            plog *= 2
            s = P.max(axis=(1, 2))
            P /= s[:, None, None]
            plog += np.log(s)
    w = (np.log(v) + vlog[:, None]) / T              # [M,S]
    mx = w.max()
    return mx + np.log(np.exp(w - mx).sum())


def _counts_from_x(x):
    counts = np.zeros((B, G), np.float32)
    for b in range(B):
        counts[b] = np.bincount(np.asarray(x[b], np.int64), minlength=G)
    return counts


def _prep_in_maps_a(counts, embed_table_f8):
    """Compact phase-1 inputs: only referenced embed rows matter for the
    counts contraction; gathering them on host (pure index marshalling)
    lets the device read ~40% of the shard.  Rows are packed with their
    counts into one partition-contiguous tensor per core."""
    shard_cols = []
    nu_max = 0
    for c in range(NC):
        cols = np.nonzero(counts[:, c * GS : (c + 1) * GS].sum(axis=0))[0]
        shard_cols.append(cols)
        nu_max = max(nu_max, len(cols))
    gsp = max(512, -(-nu_max // 256) * 256)
    kch = gsp // 256

    import ml_dtypes

    f8 = ml_dtypes.float8_e4m3fn
    in_maps = []
    for c in range(NC):
        g0 = c * GS
        cols = shard_cols[c]
        emb_pad = np.zeros((gsp, E), f8)
        emb_pad[: len(cols)] = embed_table_f8[g0 + cols]
        ctT = np.zeros((gsp, B), f8)
        # raw counts are small ints, exact in fp8
        ctT[: len(cols)] = counts[:, g0 : g0 + GS][:, cols].T.astype(f8)
        # [kch, 128, 2*E] and [kch, 128, 2*B]: row index = k*256 + 2p + r
        emb_r = emb_pad.reshape(kch, 128, 2 * E)
        ct_r = ctT.reshape(kch, 128, 2 * B)
        xa = np.concatenate([emb_r, ct_r], axis=2)   # [kch, 128, CHB]
        xa = np.ascontiguousarray(xa.transpose(1, 0, 2)).reshape(128, kch * CHB)
        in_maps.append({"xa": xa})
    return in_maps, gsp


def _prep_in_maps_b(counts, mean_emb, vocab_w_f8, vocab_b_f32):
    import ml_dtypes

    f8 = ml_dtypes.float8_e4m3fn
    bf16 = ml_dtypes.bfloat16
    # membT[p, k*32 + r*16 + m] = 32*mean_emb[m, k*256 + 2p + r]
    met = (mean_emb * 32.0).T.reshape(2, 128, 2, B)      # [k, p, r, m]
    membT = np.ascontiguousarray(met.transpose(1, 0, 2, 3).reshape(128, 4 * B)).astype(f8)
    in_maps = []
    for c in range(NC):
        g0, g1 = c * GS, (c + 1) * GS
        # vw_dr[p, gs*1000 + r*500 + j] = vocab_w[g0 + gs*500 + j, k*256 + 2p + r]
        v = vocab_w_f8[g0:g1].T.reshape(2, 128, 2, GSUB, GBLK)   # [k, p, r, gs, j]
        vk = np.ascontiguousarray(v.transpose(1, 0, 3, 2, 4)).reshape(128, 2, 8000)
        vwa = np.concatenate([membT, vk[:, 0, :4000]], axis=1)   # [128, 4064]
        vwb = np.ascontiguousarray(vk[:, 0, 4000:])              # [128, 4000]
        vwc = np.ascontiguousarray(vk[:, 1, :4000])
        vwd = np.ascontiguousarray(vk[:, 1, 4000:])
        # cv[:, :1000]: vb replicated per quadrant row; cv[:, 1000:]: counts
        # (both in the [q*32+b, h*500+j] <-> g0 + (2q+h)*500 + j layout)
        vbq = vocab_b_f32[g0:g1].reshape(4, 2 * GBLK)
        cq = counts[:, g0:g1].reshape(B, 4, 2 * GBLK).transpose(1, 0, 2)
        cv = np.zeros((128, 4 * GBLK), bf16)
        for q in range(4):
            cv[q * 32 : (q + 1) * 32, : 2 * GBLK] = vbq[q].astype(bf16)
            cv[q * 32 : q * 32 + B, 2 * GBLK :] = cq[q].astype(bf16)
        in_maps.append(
            {"vwa": vwa, "vwb": vwb, "vwc": vwc, "vwd": vwd, "cv": cv}
        )
    return in_maps


def _combine(core_outs, K):
    """Exact host-side combine of the per-(core, quadrant, b) partials."""
    sumexp = np.empty((NC, 4, B), np.float64)
    s1 = np.empty((NC, 4, B), np.float64)
    for c in range(NC):
        o = np.asarray(core_outs[c], np.float64).reshape(4, 32, 2)[:, :B]
        sumexp[c] = o[:, :, 0]
        s1[c] = o[:, :, 1]
    L = np.log(sumexp.sum(axis=(0, 1)))              # [B] (logits ~ +-0.2, safe)
    S1 = s1.sum(axis=(0, 1))
    out = K + S1 / T - L
    return out.astype(np.float32).reshape(B, 1)


def kernel(**inputs):
    import ml_dtypes

    f8 = ml_dtypes.float8_e4m3fn
    K = _hmm_const(inputs["init_dist"], inputs["transition"])
    counts = _counts_from_x(np.asarray(inputs["x"]))
    embed_table = np.asarray(inputs["embed_table"], np.float32).astype(f8)
    vocab_w = np.asarray(inputs["vocab_w"], np.float32).astype(f8)
    vocab_b = np.asarray(inputs["vocab_b"], np.float32)

    in_maps_a, gsp = _prep_in_maps_a(counts, embed_table)
    res_a = bass_utils.run_bass_kernel_spmd(
        _get_program_a(gsp), in_maps_a, core_ids=list(range(NC))
    )
    mean_emb = np.zeros((B, E), np.float64)
    for r in res_a.results:
        mean_emb += np.asarray(r["outm"], np.float64)
    mean_emb = (mean_emb / T).astype(np.float32)

    in_maps_b = _prep_in_maps_b(counts, mean_emb, vocab_w, vocab_b)
    res_b = bass_utils.run_bass_kernel_spmd(
        _get_program_b(), in_maps_b, core_ids=list(range(NC))
    )
    return _combine([r["out"] for r in res_b.results], K)
